# revision 5
# baseline (speedup 1.0000x reference)
"""Trainium2 Bass kernel for the rhyme soft-DP loss (CharLSTMLanguageModelPack).

loss[b] = softDP(sub[b]) + 10*(1 - p[b,0,tidx[b,0]])
  p = softmax(tail_logits, -1); sub[b,t,m] = sum_v p[b,t,v] * C[v, tidx[b,m]]
  softDP: dp[i,j] = softmin(dp[i-1,j]+10, dp[i,j-1]+10, dp[i-1,j-1]+sub[i-1,j-1])
  with softmin(a,b,c) = -log(e^-a + e^-b + e^-c)  (gamma=1)

Device strategy (pure data parallel over B, 1024 pairs/core):
  - Host sends softmax probabilities scaled x128 as fp8(e4m3), transposed
    lT[v, (b,t)], and gathered phon-cost packs Cc[v, b*17+j] scaled x64 fp8:
      j<16: 64*C[:, tidx[b,j]],  j=16: 64*onehot(tidx[b,0]) (first-char prob).
    Both are packed v-half-major [128, 2*W] for DoubleRow fp8 matmuls (K=256
    contracted in a single PE pass at 2 rows/cycle).
  - PE: per 8-pair tile, two "pack-4 all-pairs" DoubleRow matmuls:
      out[64=(4b,16t), 68=(4b',17j)] accumulating the full v range.
  - GpSimd: blockmask multiply (PSUM->SBUF bf16); DVE: segmented reduce over
    b' extracts diagonal blocks -> G[b,t,j] = 8192*sub (j=16 -> 8192*p_first).
  - ACT: S = exp(-G/8192) bf16; PE transposes S into PSUM layout [c,(j,g,i)].
  - exp-domain DP in bf16 (softmin becomes LINEAR):
      E[i,j] = d*(E[i-1,j]+E[i,j-1]) + S*E[i-1,j-1], d = e^-10,
    via DVE tensor_tensor_scan rows; DP reads S directly from PSUM.
  - loss = -ln(E[16,16]) + 10 - 10*G16/8192.
"""
import numpy as np
import ml_dtypes
from contextlib import ExitStack

import concourse.bass as bass
import concourse.tile as tile
from concourse import bacc, mybir
from concourse.bass_utils import run_bass_kernel_spmd

AP = bass.AP
FP32 = mybir.dt.float32
BF16 = mybir.dt.bfloat16
FP8 = mybir.dt.float8e4

N_CORES = 8
B, T, M, V = 8192, 16, 16, 256
BSH = B // N_CORES            # 1024 pairs per core
NT = BSH // 8                 # 128 tiles of 8 pairs
BT = BSH * T                  # 16384 bt columns per core
J = 17                        # 16 sub cols + 1 first-char col
CCW = BSH * J                 # 17408 cc columns per core
REG = 4 * J                   # 68 cols per pack-4 region
RPB = 7                       # regions per PSUM bank (7*68*4B = 1904 <= 2048)
INS_DEL = 10.0
D_COEF = float(np.exp(-INS_DEL))
PSCALE = 128.0                # fp8 scaling for probabilities
CSCALE = 64.0                 # fp8 scaling for phon-cost columns
GSCALE = PSCALE * CSCALE      # resulting scale on G

_cache = {}

def _ap(t, off, dims):
    """Strided free-dim view of a tile: canonical partition dim + custom free dims."""
    base = t[:]
    return AP(base.tensor, base.offset + off, [list(base.ap[0])] + [list(d) for d in dims])


def _build_nc():
    nc = bacc.Bacc("TRN2", target_bir_lowering=False, debug=False,
                   num_devices=N_CORES)
    ltq = nc.dram_tensor("ltq", [128, 2 * BT], FP8, kind="ExternalInput")
    ccq = nc.dram_tensor("ccq", [128, 2 * CCW], FP8, kind="ExternalInput")
    bmask = nc.dram_tensor("bmask", [128, RPB * REG], BF16, kind="ExternalInput")
    dmask = nc.dram_tensor("dmask", [128, 136], BF16, kind="ExternalInput")
    init0 = nc.dram_tensor("init0", [128, 136], BF16, kind="ExternalInput")
    ident = nc.dram_tensor("ident", [128, 128], BF16, kind="ExternalInput")
    identf = nc.dram_tensor("identf", [128, 128], FP32, kind="ExternalInput")
    out = nc.dram_tensor("out", [128, 8], FP32, kind="ExternalOutput")

    with tile.TileContext(nc) as tc, ExitStack() as ctx:
        P = lambda name, bufs, **kw: ctx.enter_context(
            tc.tile_pool(name=name, bufs=bufs, **kw))
        const_pool = P("const", 1)
        pt_pool = P("pt", 1)
        ps_pool = P("ps", 4, space="PSUM")
        tp_pool = P("tp", 1, space="PSUM")
        msk_pool = P("msk", 3)
        gall_pool = P("gall", 1)
        xp_pool = P("xp", 1)
        e_pool = P("e", 3)
        fin_pool = P("fin", 1)

        # constants (small; land first)
        bm = const_pool.tile([128, RPB * REG], BF16, tag="bm", name="bm")
        nc.sync.dma_start(bm[:], bmask[:])
        dm = const_pool.tile([128, 136], BF16, tag="dm", name="dm")
        nc.sync.dma_start(dm[:], dmask[:])
        i0 = const_pool.tile([128, 136], BF16, tag="i0", name="i0")
        nc.sync.dma_start(i0[:], init0[:])
        idn = const_pool.tile([128, 128], BF16, tag="idn", name="idn")
        nc.sync.dma_start(idn[:], ident[:])
        idnf = const_pool.tile([128, 128], FP32, tag="idnf", name="idnf")
        nc.sync.dma_start(idnf[:], identf[:])

        # fp8 inputs, v-half-major packed for DoubleRow
        ptq = pt_pool.tile([128, 2 * BT], FP8, tag="ptq", name="ptq")
        ccs = pt_pool.tile([128, 2 * CCW], FP8, tag="ccs", name="ccs")
        PT_CH = 2048
        CC_CH = 2176
        for k in range(8):
            po, co = k * PT_CH, k * CC_CH
            nc.sync.dma_start(ptq[:, po:po + PT_CH], ltq[:, po:po + PT_CH])
            nc.sync.dma_start(ptq[:, BT + po:BT + po + PT_CH],
                              ltq[:, BT + po:BT + po + PT_CH])
            nc.sync.dma_start(ccs[:, co:co + CC_CH], ccq[:, co:co + CC_CH])
            nc.sync.dma_start(ccs[:, CCW + co:CCW + co + CC_CH],
                              ccq[:, CCW + co:CCW + co + CC_CH])

        # G[b,t,j]: [128=(g,t), (c,j)] f32 = GSCALE * sub
        gall = gall_pool.tile([128, NT * J], FP32, tag="gall", name="gall")
        # S' = exp(-G/GSCALE) bf16, (m, c)-major
        xp = xp_pool.tile([128, NT * 16], BF16, tag="xp", name="xp")

        # matmul + extract, batches of RPB tiles
        c0 = 0
        while c0 < NT:
            nreg = min(RPB, NT - c0)
            ps = ps_pool.tile([128, 512], FP32, tag="ps", name="ps")
            for s in range(nreg):
                c = c0 + s
                # quads 0-3: one DoubleRow matmul (K=256 double-pumped);
                # ISA only allows DoubleRow dst at partition base 0.
                nc.tensor.matmul(
                    ps[0:64, REG * s:REG * s + REG],
                    _ap(ptq, c * 128, [[BT, 2], [1, 64]]),
                    _ap(ccs, (c * 8) * J, [[CCW, 2], [1, REG]]),
                    start=True, stop=True,
                    perf_mode=mybir.MatmulPerfMode.DoubleRow)
                # quads 4-7 at partition base 64: two accumulating fp8
                # matmuls (DoublePixel) over the v-halves.
                for vh in range(2):
                    nc.tensor.matmul(
                        ps[64:128, REG * s:REG * s + REG],
                        _ap(ptq, vh * BT + c * 128 + 64, [[1, 64]]),
                        _ap(ccs, vh * CCW + (c * 8 + 4) * J, [[1, REG]]),
                        start=(vh == 0), stop=(vh == 1),
                        perf_mode=mybir.MatmulPerfMode.DoublePixel)
            mk = msk_pool.tile([128, RPB * REG], BF16, tag="mk", name="mk")
            nc.vector.tensor_tensor(
                _ap(mk, 0, [[REG, nreg], [1, 4], [4, J]]),
                _ap(ps, 0, [[REG, nreg], [J, 4], [1, J]]),
                _ap(bm, 0, [[REG, nreg], [J, 4], [1, J]]),
                mybir.AluOpType.mult)
            # 4->1 quad reduce as two adds on GpSimd (it cannot touch PSUM)
            t1r = msk_pool.tile([128, RPB * 2 * J], BF16, tag="t1r", name="t1r")
            nc.gpsimd.tensor_tensor(
                _ap(t1r, 0, [[2 * J, nreg], [1, 2], [2, J]]),
                _ap(mk, 0, [[REG, nreg], [1, 2], [4, J]]),
                _ap(mk, 2, [[REG, nreg], [1, 2], [4, J]]),
                mybir.AluOpType.add)
            nc.gpsimd.tensor_tensor(
                _ap(gall, c0 * J, [[J, nreg], [1, J]]),
                _ap(t1r, 0, [[2 * J, nreg], [2, J]]),
                _ap(t1r, 1, [[2 * J, nreg], [2, J]]),
                mybir.AluOpType.add)
            nc.scalar.activation(
                _ap(xp, c0 * 16, [[16, nreg], [1, 16]]),
                _ap(gall, c0 * J, [[J, nreg], [1, 16]]),
                mybir.ActivationFunctionType.Exp, bias=0.0, scale=-1.0 / GSCALE)
            c0 += nreg

        # X2 = first-char col (j=16), f32; fct[c, g] = GSCALE*p_first(pair c*8+g)
        x2 = xp_pool.tile([128, NT], FP32, tag="x2", name="x2")
        nc.vector.tensor_copy(
            _ap(x2, 0, [[1, NT]]),
            _ap(gall, 16, [[J, NT]]))
        tpf = tp_pool.tile([128, 128], FP32, tag="tpf", name="tpf")
        nc.tensor.transpose(tpf[:], x2[:], idnf[:])
        fct = fin_pool.tile([128, 8], FP32, tag="fct", name="fct")
        nc.vector.tensor_copy(
            _ap(fct, 0, [[1, 8]]),
            _ap(tpf, 0, [[16, 8]]))

        # S rearrange: 16 transposes (x2 tile halves) into PSUM
        # tp[c, m*128 + g*16 + i] = S[pair(c,g), i, m]
        tp = tp_pool.tile([128, 16 * 128], BF16, tag="tp", name="tp")
        for m in range(16):
            for half in range(2):
                nc.tensor.transpose(
                    tp[64 * half:64 * half + 64, m * 128:(m + 1) * 128],
                    _ap(xp, m + half * 64 * 16, [[16, 64]]), idn[:])

        # DP in exp domain, bf16.  E tiles [128, (g8, jj17)]
        zt = e_pool.tile([128, 136], BF16, tag="tmp", name="tmp")
        nc.vector.memset(zt[:], 0.0)
        e_prev = e_pool.tile([128, 136], BF16, tag="e", name="e")
        nc.vector.tensor_tensor_scan(e_prev[:], dm[:], i0[:], 0.0,
                                     mybir.AluOpType.mult, mybir.AluOpType.add)
        for i in range(T):
            nc.vector.tensor_tensor(
                _ap(zt, 1, [[17, 8], [1, 16]]),
                _ap(tp, i, [[16, 8], [128, 16]]),
                _ap(e_prev, 0, [[17, 8], [1, 16]]),
                mybir.AluOpType.mult)
            a_t = e_pool.tile([128, 136], BF16, tag="a", name="a")
            nc.vector.scalar_tensor_tensor(
                a_t[:], e_prev[:], D_COEF, zt[:],
                mybir.AluOpType.mult, mybir.AluOpType.add)
            e_new = e_pool.tile([128, 136], BF16, tag="e", name="e")
            nc.vector.tensor_tensor_scan(e_new[:], dm[:], a_t[:], 0.0,
                                         mybir.AluOpType.mult, mybir.AluOpType.add)
            e_prev = e_new

        # loss = -ln(E[16,16]) + 10 - (10/GSCALE)*fct
        lne = fin_pool.tile([128, 8], FP32, tag="lne", name="lne")
        nc.scalar.activation(
            lne[:],
            _ap(e_prev, 16, [[17, 8]]),
            mybir.ActivationFunctionType.Ln, bias=0.0, scale=1.0)
        t1 = fin_pool.tile([128, 8], FP32, tag="t1", name="t1")
        nc.vector.tensor_scalar(t1[:], fct[:], -INS_DEL / GSCALE, INS_DEL,
                                mybir.AluOpType.mult, mybir.AluOpType.add)
        res = fin_pool.tile([128, 8], FP32, tag="res", name="res")
        nc.vector.tensor_tensor(res[:], t1[:], lne[:], mybir.AluOpType.subtract)
        nc.sync.dma_start(out[:], res[:])

    nc.finalize()
    return nc


def _host_prep(tail_logits, target_idx, phon_cost):
    l = np.asarray(tail_logits, dtype=np.float32)
    tidx = np.asarray(target_idx)
    C = np.asarray(phon_cost, dtype=np.float32)
    fp8 = ml_dtypes.float8_e4m3

    lmax = l.max(axis=-1, keepdims=True)
    e = np.exp(l - lmax)
    p = e / e.sum(axis=-1, keepdims=True)           # softmax probabilities
    p8 = (p * PSCALE).astype(fp8)                   # [B,T,V] fp8

    C8 = (C * CSCALE).astype(fp8)                   # [V,V] fp8
    # Cc pack: [V, B*17]; col b*17+j
    cc = np.empty((V, B * J), dtype=fp8)
    cols = cc.reshape(V, B, J)
    cols[:, :, :16] = C8[:, tidx]
    oh = np.zeros((V, B), dtype=np.float32)
    oh[tidx[:, 0], np.arange(B)] = CSCALE
    cols[:, :, 16] = oh.astype(fp8)

    # masks
    bmask = np.zeros((128, RPB * REG), dtype=np.float32)
    for pp in range(128):
        q = (pp // 16) % 4
        for s in range(RPB):
            bmask[pp, s * REG + q * J:s * REG + (q + 1) * J] = 1.0
    dmask = np.zeros((128, 136), dtype=np.float32)
    init0 = np.zeros((128, 136), dtype=np.float32)
    for g in range(8):
        dmask[:, g * 17 + 1:(g + 1) * 17] = D_COEF
        init0[:, g * 17] = 1.0
    bf = ml_dtypes.bfloat16

    in_maps = []
    for k in range(N_CORES):
        sl = slice(k * BSH, (k + 1) * BSH)
        lt = np.ascontiguousarray(
            p8[sl].transpose(2, 0, 1).reshape(V, BT))
        ccsh = cc[:, k * BSH * J:(k + 1) * BSH * J]
        in_maps.append({
            "ltq": np.ascontiguousarray(np.hstack([lt[:128], lt[128:]])),
            "ccq": np.ascontiguousarray(np.hstack([ccsh[:128], ccsh[128:]])),
            "bmask": bmask.astype(bf), "dmask": dmask.astype(bf),
            "init0": init0.astype(bf),
            "ident": np.eye(128, dtype=np.float32).astype(bf),
            "identf": np.eye(128, dtype=np.float32),
        })
    return in_maps


def kernel(tail_logits, target_idx, phon_cost):
    if "nc" not in _cache:
        _cache["nc"] = _build_nc()
    nc = _cache["nc"]
    in_maps = _host_prep(tail_logits, target_idx, phon_cost)
    res = run_bass_kernel_spmd(nc, in_maps, core_ids=list(range(N_CORES)))
    outs = [res.results[k]["out"].reshape(BSH) for k in range(N_CORES)]
    return np.concatenate(outs).astype(np.float32)


# revision 7
# speedup vs baseline: 1.0074x; 1.0074x over previous
"""Trainium2 Bass kernel for the rhyme soft-DP loss (CharLSTMLanguageModelPack).

loss[b] = softDP(sub[b]) + 10*(1 - p[b,0,tidx[b,0]])
  p = softmax(tail_logits, -1); sub[b,t,m] = sum_v p[b,t,v] * C[v, tidx[b,m]]
  softDP: dp[i,j] = softmin(dp[i-1,j]+10, dp[i,j-1]+10, dp[i-1,j-1]+sub[i-1,j-1])
  with softmin(a,b,c) = -log(e^-a + e^-b + e^-c)  (gamma=1)

Device strategy (pure data parallel over B, 1024 pairs/core):
  - Host sends softmax probabilities scaled x128 as fp8(e4m3), transposed
    lT[v, (b,t)], and gathered phon-cost packs Cc[v, b*17+j] scaled x64 fp8:
      j<16: 64*C[:, tidx[b,j]],  j=16: 64*onehot(tidx[b,0]) (first-char prob).
    Both are packed v-half-major [128, 2*W] for DoubleRow fp8 matmuls (K=256
    contracted in a single PE pass at 2 rows/cycle).
  - PE: per 8-pair tile, two "pack-4 all-pairs" DoubleRow matmuls:
      out[64=(4b,16t), 68=(4b',17j)] accumulating the full v range.
  - GpSimd: blockmask multiply (PSUM->SBUF bf16); DVE: segmented reduce over
    b' extracts diagonal blocks -> G[b,t,j] = 8192*sub (j=16 -> 8192*p_first).
  - ACT: S = exp(-G/8192) bf16; PE transposes S into PSUM layout [c,(j,g,i)].
  - exp-domain DP in bf16 (softmin becomes LINEAR):
      E[i,j] = d*(E[i-1,j]+E[i,j-1]) + S*E[i-1,j-1], d = e^-10,
    via DVE tensor_tensor_scan rows; DP reads S directly from PSUM.
  - loss = -ln(E[16,16]) + 10 - 10*G16/8192.
"""
import numpy as np
import ml_dtypes
from contextlib import ExitStack

import concourse.bass as bass
import concourse.tile as tile
from concourse import bacc, mybir
from concourse.bass_utils import run_bass_kernel_spmd

AP = bass.AP
FP32 = mybir.dt.float32
BF16 = mybir.dt.bfloat16
FP8 = mybir.dt.float8e4

N_CORES = 8
B, T, M, V = 8192, 16, 16, 256
BSH = B // N_CORES            # 1024 pairs per core
NT = BSH // 8                 # 128 tiles of 8 pairs
BT = BSH * T                  # 16384 bt columns per core
J = 17                        # 16 sub cols + 1 first-char col
CCW = BSH * J                 # 17408 cc columns per core
REG = 4 * J                   # 68 cols per pack-4 region
RPB = 7                       # regions per PSUM bank (7*68*4B = 1904 <= 2048)
INS_DEL = 10.0
D_COEF = float(np.exp(-INS_DEL))
PSCALE = 128.0                # fp8 scaling for probabilities
CSCALE = 64.0                 # fp8 scaling for phon-cost columns
GSCALE = PSCALE * CSCALE      # resulting scale on G

_cache = {}

def _ap(t, off, dims):
    """Strided free-dim view of a tile: canonical partition dim + custom free dims."""
    base = t[:]
    return AP(base.tensor, base.offset + off, [list(base.ap[0])] + [list(d) for d in dims])


def _build_nc():
    nc = bacc.Bacc("TRN2", target_bir_lowering=False, debug=False,
                   num_devices=N_CORES)
    ltq = nc.dram_tensor("ltq", [128, 2 * BT], FP8, kind="ExternalInput")
    ccq = nc.dram_tensor("ccq", [128, 2 * CCW], FP8, kind="ExternalInput")
    bmask = nc.dram_tensor("bmask", [128, RPB * REG], BF16, kind="ExternalInput")
    dmask = nc.dram_tensor("dmask", [128, 136], BF16, kind="ExternalInput")
    init0 = nc.dram_tensor("init0", [128, 136], BF16, kind="ExternalInput")
    ident = nc.dram_tensor("ident", [128, 128], BF16, kind="ExternalInput")
    identf = nc.dram_tensor("identf", [128, 128], FP32, kind="ExternalInput")
    out = nc.dram_tensor("out", [128, 8], FP32, kind="ExternalOutput")

    with tile.TileContext(nc) as tc, ExitStack() as ctx:
        P = lambda name, bufs, **kw: ctx.enter_context(
            tc.tile_pool(name=name, bufs=bufs, **kw))
        const_pool = P("const", 1)
        pt_pool = P("pt", 1)
        ps_pool = P("ps", 4, space="PSUM")
        tp_pool = P("tp", 1, space="PSUM")
        msk_pool = P("msk", 3)
        gall_pool = P("gall", 1)
        xp_pool = P("xp", 1)
        e_pool = P("e", 3)
        fin_pool = P("fin", 1)

        # fp8 inputs, v-half-major packed for DoubleRow.  4KB+ rows per DMA
        # (the DMA engines are packet-rate-bound below ~4KB/row), issued from
        # two different engine queues so descriptor generation overlaps.
        ptq = pt_pool.tile([128, 2 * BT], FP8, tag="ptq", name="ptq")
        ccs = pt_pool.tile([128, 2 * CCW], FP8, tag="ccs", name="ccs")
        PT_CH = 4096
        CC_CH = 4352
        for k in range(4):
            po, co = k * PT_CH, k * CC_CH
            nc.sync.dma_start(ptq[:, po:po + PT_CH], ltq[:, po:po + PT_CH])
            nc.sync.dma_start(ptq[:, BT + po:BT + po + PT_CH],
                              ltq[:, BT + po:BT + po + PT_CH])
            nc.scalar.dma_start(ccs[:, co:co + CC_CH], ccq[:, co:co + CC_CH])
            nc.scalar.dma_start(ccs[:, CCW + co:CCW + co + CC_CH],
                                ccq[:, CCW + co:CCW + co + CC_CH])

        # constants (small), issued from the otherwise-idle gpsimd queue
        bm = const_pool.tile([128, RPB * REG], BF16, tag="bm", name="bm")
        nc.gpsimd.dma_start(bm[:], bmask[:])
        dm = const_pool.tile([128, 136], BF16, tag="dm", name="dm")
        nc.gpsimd.dma_start(dm[:], dmask[:])
        i0 = const_pool.tile([128, 136], BF16, tag="i0", name="i0")
        nc.gpsimd.dma_start(i0[:], init0[:])
        idn = const_pool.tile([128, 128], BF16, tag="idn", name="idn")
        nc.gpsimd.dma_start(idn[:], ident[:])
        idnf = const_pool.tile([128, 128], FP32, tag="idnf", name="idnf")
        nc.gpsimd.dma_start(idnf[:], identf[:])

        # G[b,t,j]: [128=(g,t), (c,j)] f32 = GSCALE * sub
        gall = gall_pool.tile([128, NT * J], FP32, tag="gall", name="gall")
        # S' = exp(-G/GSCALE) bf16, (m, c)-major
        xp = xp_pool.tile([128, NT * 16], BF16, tag="xp", name="xp")

        # matmul + extract, batches of RPB tiles
        c0 = 0
        while c0 < NT:
            nreg = min(RPB, NT - c0)
            ps = ps_pool.tile([128, 512], FP32, tag="ps", name="ps")
            for s in range(nreg):
                c = c0 + s
                # quads 0-3: one DoubleRow matmul (K=256 double-pumped);
                # ISA only allows DoubleRow dst at partition base 0.
                nc.tensor.matmul(
                    ps[0:64, REG * s:REG * s + REG],
                    _ap(ptq, c * 128, [[BT, 2], [1, 64]]),
                    _ap(ccs, (c * 8) * J, [[CCW, 2], [1, REG]]),
                    start=True, stop=True,
                    perf_mode=mybir.MatmulPerfMode.DoubleRow)
                # quads 4-7 at partition base 64: two accumulating fp8
                # matmuls (DoublePixel) over the v-halves.
                for vh in range(2):
                    nc.tensor.matmul(
                        ps[64:128, REG * s:REG * s + REG],
                        _ap(ptq, vh * BT + c * 128 + 64, [[1, 64]]),
                        _ap(ccs, vh * CCW + (c * 8 + 4) * J, [[1, REG]]),
                        start=(vh == 0), stop=(vh == 1),
                        perf_mode=mybir.MatmulPerfMode.DoublePixel)
            mk = msk_pool.tile([128, RPB * REG], BF16, tag="mk", name="mk")
            nc.vector.tensor_tensor(
                _ap(mk, 0, [[REG, nreg], [J, 4], [1, J]]),
                _ap(ps, 0, [[REG, nreg], [J, 4], [1, J]]),
                _ap(bm, 0, [[REG, nreg], [J, 4], [1, J]]),
                mybir.AluOpType.mult)
            # 4->1 quad reduce as two adds on GpSimd (it cannot touch PSUM)
            t1r = msk_pool.tile([128, RPB * 2 * J], BF16, tag="t1r", name="t1r")
            nc.gpsimd.tensor_tensor(
                _ap(t1r, 0, [[2 * J, nreg], [J, 2], [1, J]]),
                _ap(mk, 0, [[REG, nreg], [J, 2], [1, J]]),
                _ap(mk, 2 * J, [[REG, nreg], [J, 2], [1, J]]),
                mybir.AluOpType.add)
            nc.gpsimd.tensor_tensor(
                _ap(gall, c0 * J, [[J, nreg], [1, J]]),
                _ap(t1r, 0, [[2 * J, nreg], [1, J]]),
                _ap(t1r, J, [[2 * J, nreg], [1, J]]),
                mybir.AluOpType.add)
            nc.scalar.activation(
                _ap(xp, c0 * 16, [[16, nreg], [1, 16]]),
                _ap(gall, c0 * J, [[J, nreg], [1, 16]]),
                mybir.ActivationFunctionType.Exp, bias=0.0, scale=-1.0 / GSCALE)
            c0 += nreg

        # X2 = first-char col (j=16), f32; fct[c, g] = GSCALE*p_first(pair c*8+g)
        x2 = xp_pool.tile([128, NT], FP32, tag="x2", name="x2")
        nc.vector.tensor_copy(
            _ap(x2, 0, [[1, NT]]),
            _ap(gall, 16, [[J, NT]]))
        tpf = tp_pool.tile([128, 128], FP32, tag="tpf", name="tpf")
        nc.tensor.transpose(tpf[:], x2[:], idnf[:])
        fct = fin_pool.tile([128, 8], FP32, tag="fct", name="fct")
        nc.vector.tensor_copy(
            _ap(fct, 0, [[1, 8]]),
            _ap(tpf, 0, [[16, 8]]))

        # S rearrange: 16 transposes (x2 tile halves) into PSUM
        # tp[c, m*128 + g*16 + i] = S[pair(c,g), i, m]
        tp = tp_pool.tile([128, 16 * 128], BF16, tag="tp", name="tp")
        for m in range(16):
            for half in range(2):
                nc.tensor.transpose(
                    tp[64 * half:64 * half + 64, m * 128:(m + 1) * 128],
                    _ap(xp, m + half * 64 * 16, [[16, 64]]), idn[:])

        # DP in exp domain, bf16.  E tiles [128, (g8, jj17)]
        zt = e_pool.tile([128, 136], BF16, tag="tmp", name="tmp")
        nc.vector.memset(zt[:], 0.0)
        e_prev = e_pool.tile([128, 136], BF16, tag="e", name="e")
        nc.vector.tensor_tensor_scan(e_prev[:], dm[:], i0[:], 0.0,
                                     mybir.AluOpType.mult, mybir.AluOpType.add)
        for i in range(T):
            nc.vector.tensor_tensor(
                _ap(zt, 1, [[17, 8], [1, 16]]),
                _ap(tp, i, [[16, 8], [128, 16]]),
                _ap(e_prev, 0, [[17, 8], [1, 16]]),
                mybir.AluOpType.mult)
            a_t = e_pool.tile([128, 136], BF16, tag="a", name="a")
            nc.vector.scalar_tensor_tensor(
                a_t[:], e_prev[:], D_COEF, zt[:],
                mybir.AluOpType.mult, mybir.AluOpType.add)
            e_new = e_pool.tile([128, 136], BF16, tag="e", name="e")
            nc.vector.tensor_tensor_scan(e_new[:], dm[:], a_t[:], 0.0,
                                         mybir.AluOpType.mult, mybir.AluOpType.add)
            e_prev = e_new

        # loss = -ln(E[16,16]) + 10 - (10/GSCALE)*fct
        lne = fin_pool.tile([128, 8], FP32, tag="lne", name="lne")
        nc.scalar.activation(
            lne[:],
            _ap(e_prev, 16, [[17, 8]]),
            mybir.ActivationFunctionType.Ln, bias=0.0, scale=1.0)
        t1 = fin_pool.tile([128, 8], FP32, tag="t1", name="t1")
        nc.vector.tensor_scalar(t1[:], fct[:], -INS_DEL / GSCALE, INS_DEL,
                                mybir.AluOpType.mult, mybir.AluOpType.add)
        res = fin_pool.tile([128, 8], FP32, tag="res", name="res")
        nc.vector.tensor_tensor(res[:], t1[:], lne[:], mybir.AluOpType.subtract)
        nc.sync.dma_start(out[:], res[:])

    nc.finalize()
    return nc


def _host_prep(tail_logits, target_idx, phon_cost):
    l = np.asarray(tail_logits, dtype=np.float32)
    tidx = np.asarray(target_idx)
    C = np.asarray(phon_cost, dtype=np.float32)
    fp8 = ml_dtypes.float8_e4m3

    lmax = l.max(axis=-1, keepdims=True)
    e = np.exp(l - lmax)
    p = e / e.sum(axis=-1, keepdims=True)           # softmax probabilities
    p8 = (p * PSCALE).astype(fp8)                   # [B,T,V] fp8

    C8 = (C * CSCALE).astype(fp8)                   # [V,V] fp8
    # Cc pack: [V, B*17]; col b*17+j
    cc = np.empty((V, B * J), dtype=fp8)
    cols = cc.reshape(V, B, J)
    cols[:, :, :16] = C8[:, tidx]
    oh = np.zeros((V, B), dtype=np.float32)
    oh[tidx[:, 0], np.arange(B)] = CSCALE
    cols[:, :, 16] = oh.astype(fp8)

    # masks
    bmask = np.zeros((128, RPB * REG), dtype=np.float32)
    for pp in range(128):
        q = (pp // 16) % 4
        for s in range(RPB):
            bmask[pp, s * REG + q * J:s * REG + (q + 1) * J] = 1.0
    dmask = np.zeros((128, 136), dtype=np.float32)
    init0 = np.zeros((128, 136), dtype=np.float32)
    for g in range(8):
        dmask[:, g * 17 + 1:(g + 1) * 17] = D_COEF
        init0[:, g * 17] = 1.0
    bf = ml_dtypes.bfloat16

    in_maps = []
    for k in range(N_CORES):
        sl = slice(k * BSH, (k + 1) * BSH)
        lt = np.ascontiguousarray(
            p8[sl].transpose(2, 0, 1).reshape(V, BT))
        ccsh = cc[:, k * BSH * J:(k + 1) * BSH * J]
        in_maps.append({
            "ltq": np.ascontiguousarray(np.hstack([lt[:128], lt[128:]])),
            "ccq": np.ascontiguousarray(np.hstack([ccsh[:128], ccsh[128:]])),
            "bmask": bmask.astype(bf), "dmask": dmask.astype(bf),
            "init0": init0.astype(bf),
            "ident": np.eye(128, dtype=np.float32).astype(bf),
            "identf": np.eye(128, dtype=np.float32),
        })
    return in_maps


def kernel(tail_logits, target_idx, phon_cost):
    if "nc" not in _cache:
        _cache["nc"] = _build_nc()
    nc = _cache["nc"]
    in_maps = _host_prep(tail_logits, target_idx, phon_cost)
    res = run_bass_kernel_spmd(nc, in_maps, core_ids=list(range(N_CORES)))
    outs = [res.results[k]["out"].reshape(BSH) for k in range(N_CORES)]
    return np.concatenate(outs).astype(np.float32)


# revision 8
# speedup vs baseline: 1.4572x; 1.4465x over previous
"""Trainium2 Bass kernel for the rhyme soft-DP loss (CharLSTMLanguageModelPack).

loss[b] = softDP(sub[b]) + 10*(1 - p[b,0,tidx[b,0]])
  p = softmax(tail_logits, -1); sub[b,t,m] = sum_v p[b,t,v] * C[v, tidx[b,m]]
  softDP: dp[i,j] = softmin(dp[i-1,j]+10, dp[i,j-1]+10, dp[i-1,j-1]+sub[i-1,j-1])
  with softmin(a,b,c) = -log(e^-a + e^-b + e^-c)  (gamma=1)

Device strategy (pure data parallel over B, 1024 pairs/core):
  - Host sends softmax probabilities scaled x128 as fp8(e4m3), transposed
    lT[v, (b,t)], and gathered phon-cost packs Cc[v, b*17+j] scaled x64 fp8:
      j<16: 64*C[:, tidx[b,j]],  j=16: 64*onehot(tidx[b,0]) (first-char prob).
    Both are packed v-half-major [128, 2*W] for DoubleRow fp8 matmuls (K=256
    contracted in a single PE pass at 2 rows/cycle).
  - PE: per 8-pair tile, two "pack-4 all-pairs" DoubleRow matmuls:
      out[64=(4b,16t), 68=(4b',17j)] accumulating the full v range.
  - GpSimd: blockmask multiply (PSUM->SBUF bf16); DVE: segmented reduce over
    b' extracts diagonal blocks -> G[b,t,j] = 8192*sub (j=16 -> 8192*p_first).
  - ACT: S = exp(-G/8192) bf16; PE transposes S into PSUM layout [c,(j,g,i)].
  - exp-domain DP in bf16 (softmin becomes LINEAR):
      E[i,j] = d*(E[i-1,j]+E[i,j-1]) + S*E[i-1,j-1], d = e^-10,
    via DVE tensor_tensor_scan rows; DP reads S directly from PSUM.
  - loss = -ln(E[16,16]) + 10 - 10*G16/8192.
"""
import numpy as np
import ml_dtypes
from contextlib import ExitStack

import concourse.bass as bass
import concourse.tile as tile
from concourse import bacc, mybir
from concourse.bass_utils import run_bass_kernel_spmd

AP = bass.AP
FP32 = mybir.dt.float32
BF16 = mybir.dt.bfloat16
FP8 = mybir.dt.float8e4

N_CORES = 8
B, T, M, V = 8192, 16, 16, 256
BSH = B // N_CORES            # 1024 pairs per core
NT = BSH // 8                 # 128 tiles of 8 pairs
BT = BSH * T                  # 16384 bt columns per core
J = 17                        # 16 sub cols + 1 first-char col
CCW = BSH * J                 # 17408 cc columns per core
REG = 4 * J                   # 68 cols per pack-4 region
RPB = 7                       # regions per PSUM bank (7*68*4B = 1904 <= 2048)
INS_DEL = 10.0
D_COEF = float(np.exp(-INS_DEL))
PSCALE = 128.0                # fp8 scaling for probabilities
CSCALE = 64.0                 # fp8 scaling for phon-cost columns
GSCALE = PSCALE * CSCALE      # resulting scale on G

_cache = {}

def _ap(t, off, dims):
    """Strided free-dim view of a tile: canonical partition dim + custom free dims."""
    base = t[:]
    return AP(base.tensor, base.offset + off, [list(base.ap[0])] + [list(d) for d in dims])


def _build_nc():
    nc = bacc.Bacc("TRN2", target_bir_lowering=False, debug=False,
                   num_devices=N_CORES)
    ltq = nc.dram_tensor("ltq", [128, 2 * BT], FP8, kind="ExternalInput")
    ccq = nc.dram_tensor("ccq", [128, 2 * CCW], FP8, kind="ExternalInput")
    bmask = nc.dram_tensor("bmask", [128, RPB * REG], BF16, kind="ExternalInput")
    dmask = nc.dram_tensor("dmask", [128, 136], BF16, kind="ExternalInput")
    init0 = nc.dram_tensor("init0", [128, 136], BF16, kind="ExternalInput")
    ident = nc.dram_tensor("ident", [128, 128], BF16, kind="ExternalInput")
    identf = nc.dram_tensor("identf", [128, 128], FP32, kind="ExternalInput")
    out = nc.dram_tensor("out", [128, 8], FP32, kind="ExternalOutput")

    with tile.TileContext(nc) as tc, ExitStack() as ctx:
        P = lambda name, bufs, **kw: ctx.enter_context(
            tc.tile_pool(name=name, bufs=bufs, **kw))
        const_pool = P("const", 1)
        pt_pool = P("pt", 1)
        ps_pool = P("ps", 4, space="PSUM")
        tp_pool = P("tp", 1, space="PSUM")
        msk_pool = P("msk", 3)
        gall_pool = P("gall", 1)
        xp_pool = P("xp", 1)
        e_pool = P("e", 3)
        fin_pool = P("fin", 1)

        # fp8 inputs, v-half-major packed for DoubleRow.  4KB+ rows per DMA
        # (the DMA engines are packet-rate-bound below ~4KB/row), issued from
        # two different engine queues so descriptor generation overlaps.
        ptq = pt_pool.tile([128, 2 * BT], FP8, tag="ptq", name="ptq")
        ccs = pt_pool.tile([128, 2 * CCW], FP8, tag="ccs", name="ccs")
        PT_CH = 4096
        CC_CH = 4352
        for k in range(4):
            po, co = k * PT_CH, k * CC_CH
            nc.sync.dma_start(ptq[:, po:po + PT_CH], ltq[:, po:po + PT_CH])
            nc.sync.dma_start(ptq[:, BT + po:BT + po + PT_CH],
                              ltq[:, BT + po:BT + po + PT_CH])
            nc.scalar.dma_start(ccs[:, co:co + CC_CH], ccq[:, co:co + CC_CH])
            nc.scalar.dma_start(ccs[:, CCW + co:CCW + co + CC_CH],
                                ccq[:, CCW + co:CCW + co + CC_CH])

        # constants (small), issued from the otherwise-idle gpsimd queue
        bm = const_pool.tile([128, RPB * REG], BF16, tag="bm", name="bm")
        nc.gpsimd.dma_start(bm[:], bmask[:])
        dm = const_pool.tile([128, 136], BF16, tag="dm", name="dm")
        nc.gpsimd.dma_start(dm[:], dmask[:])
        i0 = const_pool.tile([128, 136], BF16, tag="i0", name="i0")
        nc.gpsimd.dma_start(i0[:], init0[:])
        idn = const_pool.tile([128, 128], BF16, tag="idn", name="idn")
        nc.gpsimd.dma_start(idn[:], ident[:])
        idnf = const_pool.tile([128, 128], FP32, tag="idnf", name="idnf")
        nc.gpsimd.dma_start(idnf[:], identf[:])

        # G[b,t,j]: [128=(g,t), (c,j)] f32 = GSCALE * sub
        gall = gall_pool.tile([128, NT * J], FP32, tag="gall", name="gall")
        # S' = exp(-G/GSCALE) bf16, (m, c)-major
        xp = xp_pool.tile([128, NT * 16], BF16, tag="xp", name="xp")

        # matmul + extract, batches of RPB tiles
        c0 = 0
        while c0 < NT:
            nreg = min(RPB, NT - c0)
            ps = ps_pool.tile([128, 512], FP32, tag="ps", name="ps")
            # group by perf mode: alternating modes per tile stalls the PE.
            # quads 0-3: one DoubleRow matmul each (K=256 double-pumped);
            # ISA only allows DoubleRow dst at partition base 0.
            for s in range(nreg):
                c = c0 + s
                nc.tensor.matmul(
                    ps[0:64, REG * s:REG * s + REG],
                    _ap(ptq, c * 128, [[BT, 2], [1, 64]]),
                    _ap(ccs, (c * 8) * J, [[CCW, 2], [1, REG]]),
                    start=True, stop=True,
                    perf_mode=mybir.MatmulPerfMode.DoubleRow)
            # quads 4-7 at partition base 64: two accumulating plain fp8
            # matmuls over the v-halves.
            for s in range(nreg):
                c = c0 + s
                for vh in range(2):
                    nc.tensor.matmul(
                        ps[64:128, REG * s:REG * s + REG],
                        _ap(ptq, vh * BT + c * 128 + 64, [[1, 64]]),
                        _ap(ccs, vh * CCW + (c * 8 + 4) * J, [[1, REG]]),
                        start=(vh == 0), stop=(vh == 1))
            mk = msk_pool.tile([128, RPB * REG], BF16, tag="mk", name="mk")
            nc.vector.tensor_tensor(
                _ap(mk, 0, [[REG, nreg], [J, 4], [1, J]]),
                _ap(ps, 0, [[REG, nreg], [J, 4], [1, J]]),
                _ap(bm, 0, [[REG, nreg], [J, 4], [1, J]]),
                mybir.AluOpType.mult)
            # 4->1 quad reduce as two adds on GpSimd (it cannot touch PSUM)
            t1r = msk_pool.tile([128, RPB * 2 * J], BF16, tag="t1r", name="t1r")
            nc.gpsimd.tensor_tensor(
                _ap(t1r, 0, [[2 * J, nreg], [J, 2], [1, J]]),
                _ap(mk, 0, [[REG, nreg], [J, 2], [1, J]]),
                _ap(mk, 2 * J, [[REG, nreg], [J, 2], [1, J]]),
                mybir.AluOpType.add)
            nc.gpsimd.tensor_tensor(
                _ap(gall, c0 * J, [[J, nreg], [1, J]]),
                _ap(t1r, 0, [[2 * J, nreg], [1, J]]),
                _ap(t1r, J, [[2 * J, nreg], [1, J]]),
                mybir.AluOpType.add)
            nc.scalar.activation(
                _ap(xp, c0 * 16, [[16, nreg], [1, 16]]),
                _ap(gall, c0 * J, [[J, nreg], [1, 16]]),
                mybir.ActivationFunctionType.Exp, bias=0.0, scale=-1.0 / GSCALE)
            c0 += nreg

        # X2 = first-char col (j=16), f32; fct[c, g] = GSCALE*p_first(pair c*8+g)
        x2 = xp_pool.tile([128, NT], FP32, tag="x2", name="x2")
        nc.vector.tensor_copy(
            _ap(x2, 0, [[1, NT]]),
            _ap(gall, 16, [[J, NT]]))
        tpf = tp_pool.tile([128, 128], FP32, tag="tpf", name="tpf")
        nc.tensor.transpose(tpf[:], x2[:], idnf[:])
        fct = fin_pool.tile([128, 8], FP32, tag="fct", name="fct")
        nc.vector.tensor_copy(
            _ap(fct, 0, [[1, 8]]),
            _ap(tpf, 0, [[16, 8]]))

        # S rearrange: 16 transposes (x2 tile halves) into PSUM
        # tp[c, m*128 + g*16 + i] = S[pair(c,g), i, m]
        tp = tp_pool.tile([128, 16 * 128], BF16, tag="tp", name="tp")
        for m in range(16):
            for half in range(2):
                nc.tensor.transpose(
                    tp[64 * half:64 * half + 64, m * 128:(m + 1) * 128],
                    _ap(xp, m + half * 64 * 16, [[16, 64]]), idn[:])

        # DP in exp domain, bf16.  E tiles [128, (g8, jj17)]
        zt = e_pool.tile([128, 136], BF16, tag="tmp", name="tmp")
        nc.vector.memset(zt[:], 0.0)
        e_prev = e_pool.tile([128, 136], BF16, tag="e", name="e")
        nc.vector.tensor_tensor_scan(e_prev[:], dm[:], i0[:], 0.0,
                                     mybir.AluOpType.mult, mybir.AluOpType.add)
        for i in range(T):
            nc.vector.tensor_tensor(
                _ap(zt, 1, [[17, 8], [1, 16]]),
                _ap(tp, i, [[16, 8], [128, 16]]),
                _ap(e_prev, 0, [[17, 8], [1, 16]]),
                mybir.AluOpType.mult)
            a_t = e_pool.tile([128, 136], BF16, tag="a", name="a")
            nc.vector.scalar_tensor_tensor(
                a_t[:], e_prev[:], D_COEF, zt[:],
                mybir.AluOpType.mult, mybir.AluOpType.add)
            e_new = e_pool.tile([128, 136], BF16, tag="e", name="e")
            nc.vector.tensor_tensor_scan(e_new[:], dm[:], a_t[:], 0.0,
                                         mybir.AluOpType.mult, mybir.AluOpType.add)
            e_prev = e_new

        # loss = -ln(E[16,16]) + 10 - (10/GSCALE)*fct
        lne = fin_pool.tile([128, 8], FP32, tag="lne", name="lne")
        nc.scalar.activation(
            lne[:],
            _ap(e_prev, 16, [[17, 8]]),
            mybir.ActivationFunctionType.Ln, bias=0.0, scale=1.0)
        t1 = fin_pool.tile([128, 8], FP32, tag="t1", name="t1")
        nc.vector.tensor_scalar(t1[:], fct[:], -INS_DEL / GSCALE, INS_DEL,
                                mybir.AluOpType.mult, mybir.AluOpType.add)
        res = fin_pool.tile([128, 8], FP32, tag="res", name="res")
        nc.vector.tensor_tensor(res[:], t1[:], lne[:], mybir.AluOpType.subtract)
        nc.sync.dma_start(out[:], res[:])

    nc.finalize()
    return nc


def _host_prep(tail_logits, target_idx, phon_cost):
    l = np.asarray(tail_logits, dtype=np.float32)
    tidx = np.asarray(target_idx)
    C = np.asarray(phon_cost, dtype=np.float32)
    fp8 = ml_dtypes.float8_e4m3

    lmax = l.max(axis=-1, keepdims=True)
    e = np.exp(l - lmax)
    p = e / e.sum(axis=-1, keepdims=True)           # softmax probabilities
    p8 = (p * PSCALE).astype(fp8)                   # [B,T,V] fp8

    C8 = (C * CSCALE).astype(fp8)                   # [V,V] fp8
    # Cc pack: [V, B*17]; col b*17+j
    cc = np.empty((V, B * J), dtype=fp8)
    cols = cc.reshape(V, B, J)
    cols[:, :, :16] = C8[:, tidx]
    oh = np.zeros((V, B), dtype=np.float32)
    oh[tidx[:, 0], np.arange(B)] = CSCALE
    cols[:, :, 16] = oh.astype(fp8)

    # masks
    bmask = np.zeros((128, RPB * REG), dtype=np.float32)
    for pp in range(128):
        q = (pp // 16) % 4
        for s in range(RPB):
            bmask[pp, s * REG + q * J:s * REG + (q + 1) * J] = 1.0
    dmask = np.zeros((128, 136), dtype=np.float32)
    init0 = np.zeros((128, 136), dtype=np.float32)
    for g in range(8):
        dmask[:, g * 17 + 1:(g + 1) * 17] = D_COEF
        init0[:, g * 17] = 1.0
    bf = ml_dtypes.bfloat16

    in_maps = []
    for k in range(N_CORES):
        sl = slice(k * BSH, (k + 1) * BSH)
        lt = np.ascontiguousarray(
            p8[sl].transpose(2, 0, 1).reshape(V, BT))
        ccsh = cc[:, k * BSH * J:(k + 1) * BSH * J]
        in_maps.append({
            "ltq": np.ascontiguousarray(np.hstack([lt[:128], lt[128:]])),
            "ccq": np.ascontiguousarray(np.hstack([ccsh[:128], ccsh[128:]])),
            "bmask": bmask.astype(bf), "dmask": dmask.astype(bf),
            "init0": init0.astype(bf),
            "ident": np.eye(128, dtype=np.float32).astype(bf),
            "identf": np.eye(128, dtype=np.float32),
        })
    return in_maps


def kernel(tail_logits, target_idx, phon_cost):
    if "nc" not in _cache:
        _cache["nc"] = _build_nc()
    nc = _cache["nc"]
    in_maps = _host_prep(tail_logits, target_idx, phon_cost)
    res = run_bass_kernel_spmd(nc, in_maps, core_ids=list(range(N_CORES)))
    outs = [res.results[k]["out"].reshape(BSH) for k in range(N_CORES)]
    return np.concatenate(outs).astype(np.float32)
